# revision 21
# baseline (speedup 1.0000x reference)
"""Trainium2 Bass kernel for nn_DCGN_78967268704510.

Math: the reference's get_adjacent() builds a diagonal matrix (the faithful
buggy triple loop zeroes every off-diagonal), adds I, then symmetric-
normalizes; for a diagonal matrix D^-1/2 A D^-1/2 == I exactly (to fp32
ulps).  attn_pool feeds only get_adjacent, so the whole network collapses
to two fused stages:

  h   = leaky( (sum_p x[:,4s+p,:] * conv1_w[p,:]) @ prop1_W + prop1_B )
  out = leaky( (sum_p h[:,4t+p,:] * conv2_w[p,:]) @ prop2_W + prop2_B )

Sharding: pure data parallel, batch 64 -> 8 cores x 8 batches each.

v2.1 design (PE-throughput + DMA-overlap oriented):
  - single SP DMA ring with hand-interleaved order: consts/w1rep/b1rep,
    x(0), x(1), w1p[0:8], x(2), w1p[8:16], x(3), stage-2 weights,
    x(4..7) -- every tensor arrives just before its consumer needs it,
    so the PE never starves and the HWDGE FIFO never head-of-line
    blocks.
  - x arrives as one 2MB DMA per (batch, f-half) in a [128, q*4096+f]
    gathered layout: one SBUF tile per half-batch, 4-deep pool.
  - ~28 warmup matmuls keep the PE busy from t~0 so the HAM clock gate
    opens early and never re-throttles (cold PE halves matmul rate).
  - pool selectors stored contiguously per q (strided gpad windows cost
    ~300ns LDWEIGHTS vs ~190ns contiguous).
  - conv1 scale split 5:3 across DVE and GpSimd (measured rates ~1.4 vs
    ~2.8 us per [128,1024] under DMA load).
  - software pipeline: mm1(b-1) / pool(b) / stage2(b-1) / transpose(b)
    interleaved on the PE; scale(b+2) emitted a full iteration ahead.

All replicated weight/bias tiles are host-built exact fp32 (replicating
on-chip through fp32r matmuls costs ~5e-4 relative rounding and blows
the 2e-2 gate: measured rel err 1.93e-2 -> 2.21e-2).
"""
import sys

if '/opt/trn_rl_repo' not in sys.path:
    sys.path.insert(0, '/opt/trn_rl_repo')

import numpy as np

import concourse.bass as bass
import concourse.mybir as mybir
import concourse.tile as tile
from concourse.bass_utils import run_bass_kernel_spmd
from concourse.vector_clock import ScopedClock

N_CORES = 8
B, N, F, HID, NCLASS, P = 64, 512, 2048, 1100, 512, 4
BPC = B // N_CORES          # 8 batches per core
S = N // P                  # 128 stage-1 nodes
T = S // P                  # 32 stage-2 nodes
FT = F // 128               # 16 f-tiles
JT = (HID + 127) // 128     # 9 j-tiles, last has 76 rows
JLAST = HID - 128 * (JT - 1)
MM1_CHUNKS = (384, 384, 332)   # all >=256 so float32r runs at 1 cyc/row

FP32 = mybir.dt.float32
F32R = mybir.dt.float32r


class PatchedTileContext(tile.TileContext):
    """This container's walrus refuses ANY instruction carrying >1 sync
    wait (the TPB EVENTS struct has a single wait slot and the codegen
    won't split).  Split every multi-wait instruction into single-wait
    same-engine nops followed by the instruction with its last wait."""

    def _split_waits(self, inst):
        si = inst.sync_info
        waits = list(si.on_wait) if si and si.on_wait else []
        if len(waits) <= 1:
            return
        for w in waits[:-1]:
            nop = mybir.InstNoOp(
                name=self.nc.get_next_instruction_name(), ins=[], outs=[]
            )
            nop.engine = inst.engine
            nop.sync_info = mybir.SyncInfo(on_wait=[w], on_update=[])
            nop.bass_nofuse = True
            self._add_instruction(nop)
        inst.sync_info = mybir.SyncInfo(
            on_wait=[waits[-1]], on_update=list(si.on_update or [])
        )

    def _commit_instruction(self, inst, lazy_reg_writes=True):
        if inst.engine != mybir.EngineType.Unassigned:
            self._split_waits(inst)
        return super()._commit_instruction(inst, lazy_reg_writes)

    def _drain_and_barrier(self, tick_clock, wait_clock):
        probe = self.nc.sync.nop()
        wait_clock.add_sem_waits(
            probe.ins, ScopedClock({None: tick_clock.global_clock})
        )
        si = probe.ins.sync_info
        waits = list(si.on_wait) if si and si.on_wait else []
        if si and waits:
            probe.ins.sync_info = mybir.SyncInfo(
                on_wait=waits[:1], on_update=list(si.on_update or [])
            )
        for w in waits[1:]:
            n2 = self.nc.sync.nop()
            n2.ins.sync_info = mybir.SyncInfo(on_wait=[w], on_update=[])
        self.nc.sync.drain()
        self.nc.all_engine_barrier()
        assert self.sems is not None
        popped = self.nc._tile_sem_poison_stack.pop()
        assert popped is self._sem_poison
        self.nc.clear_and_free_semaphores(list(self.sems.allocated().values()))
        self.nc.all_engine_barrier()


def build_nc():
    nc = bass.Bass()
    xs_d = nc.dram_tensor('xs', [BPC, N, F], F32R, kind='ExternalInput')
    gq_d = nc.dram_tensor('gq', [128, 512], F32R, kind='ExternalInput')
    g2_d = nc.dram_tensor('g2', [128, 32], F32R, kind='ExternalInput')
    id_d = nc.dram_tensor('ident', [128, 128], F32R, kind='ExternalInput')
    w1rep_d = nc.dram_tensor('w1rep', [128, F], FP32, kind='ExternalInput')
    b1rep_d = nc.dram_tensor('b1rep', [128, HID], FP32, kind='ExternalInput')
    w2rep_d = nc.dram_tensor('w2rep', [128, HID], FP32, kind='ExternalInput')
    b2rep_d = nc.dram_tensor('b2rep', [128, NCLASS], FP32,
                             kind='ExternalInput')
    w1p_d = nc.dram_tensor('w1p', [F, HID], F32R, kind='ExternalInput')
    w2p_d = nc.dram_tensor('w2p', [HID, NCLASS], F32R, kind='ExternalInput')
    y_d = nc.dram_tensor('y', [BPC, T, NCLASS], FP32, kind='ExternalOutput')
    y_flat = y_d.rearrange('b t c -> (b t) c')   # [256, 512]

    with PatchedTileContext(nc) as tc:
        with (
            tc.tile_pool(name='wpool', bufs=1) as wpool,
            tc.tile_pool(name='cpool', bufs=4) as cpool,
            tc.tile_pool(name='xcpool', bufs=2) as xcpool,
            tc.tile_pool(name='xcTpool', bufs=3) as xcTpool,
            tc.tile_pool(name='h2pool', bufs=1) as h2pool,
            tc.tile_pool(name='hcTpool', bufs=1) as hcTpool,
            tc.tile_pool(name='opool', bufs=1) as opool,
            tc.tile_pool(name='pbig', bufs=2, space='PSUM') as pbigpool,
            tc.tile_pool(name='ph', bufs=3, space='PSUM') as phpool,
            tc.tile_pool(name='p2', bufs=1, space='PSUM') as p2pool,
        ):
            # ---- DMA order group 1: tiny consts + stage-1 scale/bias ----
            gq = wpool.tile([128, 512], F32R, tag='gq')
            nc.sync.dma_start(out=gq[:], in_=gq_d[:])
            g2 = wpool.tile([128, 32], F32R, tag='g2')
            nc.sync.dma_start(out=g2[:], in_=g2_d[:])
            ident = wpool.tile([128, 128], F32R, tag='ident')
            nc.sync.dma_start(out=ident[:], in_=id_d[:])
            w1rep = wpool.tile([128, F], FP32, tag='w1rep')
            nc.sync.dma_start(out=w1rep[:], in_=w1rep_d[:])

            # ---- x loads: one 2MB DMA per (batch, f-half) ----
            cs = {}

            def load_ch(b, h):
                src = xs_d[b].rearrange('(q p) (h f) -> p h q f', p=128, h=2)
                c = cpool.tile([128, 4096], F32R, tag='c',
                               name=f'c_{b}_{h}')
                nc.sync.dma_start(
                    out=c.rearrange('p (q f) -> p q f', q=4),
                    in_=src[:, h],
                )
                cs[(b, h)] = c

            def load_c(b):
                load_ch(b, 0)
                load_ch(b, 1)

            load_c(0)

            # b1rep/w2rep are first needed by mm1(0)'s epilogue (~t=30us);
            # issue after x(0) so pool(0) starts as early as possible
            b1rep = wpool.tile([128, HID], FP32, tag='b1rep')
            nc.sync.dma_start(out=b1rep[:], in_=b1rep_d[:])
            w2rep = wpool.tile([128, HID], FP32, tag='w2rep')
            nc.sync.dma_start(out=w2rep[:], in_=w2rep_d[:])

            load_c(1)

            # ---- DMA order group 2: first chunk of w1p ----
            w1ps = []
            for k in range(FT):
                w1ps.append(wpool.tile([128, HID], F32R, tag=f'w1p{k}',
                                       name=f'w1p{k}'))

            def load_w1p(k0, k1):
                for k in range(k0, k1):
                    nc.sync.dma_start(
                        out=w1ps[k][:], in_=w1p_d[k * 128:(k + 1) * 128, :]
                    )

            load_w1p(0, 6)

            # stage-2 weights (streamed across iterations 1-2; mm2(0) at
            # iteration 4 consumes w2p m-tiles in order)
            b2rep = wpool.tile([128, NCLASS], FP32, tag='b2rep')
            w2p = wpool.tile([128, JT * NCLASS], F32R, tag='w2p')

            def load_w2p(m0, m1):
                for m in range(m0, m1):
                    rows = 128 if m < JT - 1 else JLAST
                    nc.sync.dma_start(
                        out=w2p[0:rows, m * NCLASS:(m + 1) * NCLASS],
                        in_=w2p_d[m * 128:m * 128 + rows, :],
                    )

            # ---- PE warmup: dummy matmuls during the initial DMA wait so
            #      the HAM clock gate reaches K=8/8 before batch 0 ----
            for w in range(4):
                warm_ps = p2pool.tile([128, 512], FP32, tag='p2',
                                      name=f'warm{w}')
                for i in range(7):
                    nc.tensor.matmul(warm_ps[:], ident[:], gq[:],
                                     start=(i == 0), stop=(i == 6))

            # ---- per-batch stages ----
            # ALL elementwise work runs on DVE: any concurrent GpSimd
            # tensor op degrades DVE from ~1.2us to ~3.4us per [128,1024]
            # (measured), so GpSimd's 3.3us/op "help" is net-negative.

            def scale_c(b):
                for h in range(2):
                    cv = cs[(b, h)].rearrange('p (q f) -> p q f', q=4)
                    wsl = w1rep[:, h * 1024:(h + 1) * 1024]
                    for q in range(4):
                        nc.vector.tensor_mul(cv[:, q], cv[:, q], wsl)

            xcs = {}
            xcTs = {}
            h2s = {}
            hcT = [None, None]

            def pool_half(b, h):
                """pool f-half h of batch b -> xc[b][h] (SBUF [128,1024])."""
                cv = cs[(b, h)].rearrange('p (q f) -> p q f', q=4)
                pp = pbigpool.tile([128, 1024], FP32, tag='pbig',
                                   name=f'pp_{b}_{h}')
                for c2 in range(2):
                    for q in range(4):
                        nc.tensor.matmul(
                            pp[:, 512 * c2:512 * (c2 + 1)],
                            gq[:, 128 * q:128 * (q + 1)],
                            cv[:, q, 512 * c2:512 * (c2 + 1)],
                            start=(q == 0), stop=(q == 3),
                        )
                xc = xcpool.tile([128, 1024], F32R, tag='xc',
                                 name=f'xc_{b}_{h}')
                nc.scalar.copy(out=xc[:], in_=pp[:])
                xcs[(b, h)] = xc

            def transpose_half(b, h):
                xc = xcs.pop((b, h))
                pt = pbigpool.tile([128, 1024], F32R, tag='pbig',
                                   name=f'pt_{b}_{h}')
                for kk in range(8):
                    nc.tensor.transpose(
                        pt[:, 128 * kk:128 * (kk + 1)],
                        xc[:, kk * 128:(kk + 1) * 128],
                        ident[:],
                    )
                xcT = xcTpool.tile([128, 1024], F32R, tag='xcT',
                                   name=f'xcT_{b}_{h}')
                nc.scalar.copy(out=xcT[:], in_=pt[:])
                xcTs[(b, h)] = xcT

            def mm1(b):
                """mm1 + bias + leaky + conv2-scale for batch b."""
                xcT0 = xcTs.pop((b, 0))
                xcT1 = xcTs.pop((b, 1))
                xcT = (xcT0, xcT1)
                h2 = h2pool.tile([128, HID], F32R, tag='h2', name=f'h2_{b}')
                c0 = 0
                for cn in MM1_CHUNKS:
                    pht = phpool.tile([128, cn], FP32, tag='ph',
                                      name=f'ph_{b}_{c0}')
                    for k in range(FT):
                        nc.tensor.matmul(
                            pht[:],
                            xcT[k // 8][:, (k % 8) * 128:(k % 8 + 1) * 128],
                            w1ps[k][:, c0:c0 + cn],
                            start=(k == 0), stop=(k == FT - 1),
                        )
                    nc.vector.tensor_add(
                        h2[:, c0:c0 + cn], pht[:], b1rep[:, c0:c0 + cn]
                    )
                    nc.scalar.activation(
                        h2[:, c0:c0 + cn], h2[:, c0:c0 + cn],
                        mybir.ActivationFunctionType.Lrelu, alpha=0.01,
                    )
                    c0 += cn
                nc.vector.tensor_mul(h2[:], h2[:], w2rep[:])
                h2s[b] = h2

            def stage2(b):
                """stage-2 pool-transpose + (every 4th) mm2 + store."""
                h2 = h2s.pop(b)
                pt2 = p2pool.tile([128, JT * T], FP32, tag='p2',
                                  name=f'pt2_{b}')
                for m in range(JT):
                    rows = 128 if m < JT - 1 else JLAST
                    nc.tensor.matmul(
                        pt2[0:rows, m * T:(m + 1) * T],
                        h2[:, m * 128:m * 128 + rows],
                        g2[:],
                        start=True, stop=True,
                    )
                g, bg = divmod(b, 4)
                if bg == 0:
                    hcT[g] = hcTpool.tile(
                        [128, JT * 128], F32R, tag='hcT', name=f'hcT{g}'
                    )
                dst = hcT[g].rearrange('p (m c) -> p m c', m=JT)[
                    :, :, 32 * bg:32 * (bg + 1)
                ]
                src = pt2[:].rearrange('p (m c) -> p m c', m=JT)
                # region-exact: rows [JLAST:128] of the last j-block are
                # never written by the pt2 matmuls
                nc.scalar.copy(out=dst[:, 0:JT - 1], in_=src[:, 0:JT - 1])
                nc.scalar.copy(out=dst[0:JLAST, JT - 1:JT],
                               in_=src[0:JLAST, JT - 1:JT])

                if bg == 3:
                    po = p2pool.tile([128, NCLASS], FP32, tag='p2',
                                     name=f'po_{g}')
                    for m in range(JT):
                        rows = 128 if m < JT - 1 else JLAST
                        nc.tensor.matmul(
                            po[:],
                            hcT[g][0:rows, m * 128:(m + 1) * 128],
                            w2p[0:rows, m * NCLASS:(m + 1) * NCLASS],
                            start=(m == 0), stop=(m == JT - 1),
                        )
                    ob = opool.tile([128, NCLASS], FP32, tag='ob',
                                    name=f'ob_{g}')
                    nc.vector.tensor_add(ob[:], po[:], b2rep[:])
                    nc.scalar.activation(
                        ob[:], ob[:],
                        mybir.ActivationFunctionType.Lrelu, alpha=0.01,
                    )
                    nc.sync.dma_start(
                        out=y_flat[128 * g:128 * (g + 1), :], in_=ob[:]
                    )

            # ---- software pipeline ----
            scale_c(0)
            scale_c(1)
            for b in range(BPC):
                if b == 0:
                    # interleave the rest of w1p with x(2) at half-batch
                    # granularity so neither stream starves its consumer
                    load_ch(2, 0)
                    load_w1p(6, 11)
                    load_ch(2, 1)
                    load_w1p(11, FT)
                elif b + 2 < BPC:
                    load_c(b + 2)
                if b == 1:
                    load_w2p(0, 5)
                if b == 2:
                    load_w2p(5, JT)
                    nc.sync.dma_start(out=b2rep[:], in_=b2rep_d[:])
                if b >= 1:
                    mm1(b - 1)
                pool_half(b, 0)
                pool_half(b, 1)
                transpose_half(b, 0)
                transpose_half(b, 1)
                if b >= 1:
                    stage2(b - 1)
                if b + 2 < BPC:
                    scale_c(b + 2)
                cs.pop((b, 0))
                cs.pop((b, 1))
            mm1(BPC - 1)
            stage2(BPC - 1)
    return nc


def _host_consts(conv1_w, pool1_w, pool1_b, prop1_W, prop1_B,
                 conv2_w, pool2_w, pool2_b, prop2_W, prop2_B):
    f32 = lambda a: np.ascontiguousarray(np.asarray(a, dtype=np.float32))
    gq = np.zeros((128, 512), dtype=np.float32)
    n = np.arange(128)
    for q in range(4):
        gq[n, 128 * q + 32 * q + n // 4] = 1.0
    g2 = np.zeros((128, 32), dtype=np.float32)
    g2[n, n // 4] = 1.0
    return {
        'gq': gq,
        'g2': g2,
        'ident': np.eye(128, dtype=np.float32),
        'w1rep': f32(np.tile(np.asarray(conv1_w), (32, 1))),
        'b1rep': f32(np.broadcast_to(np.asarray(prop1_B), (128, HID))),
        'w2rep': f32(np.tile(np.asarray(conv2_w), (32, 1))),
        'b2rep': f32(np.broadcast_to(np.asarray(prop2_B), (128, NCLASS))),
        'w1p': f32(prop1_W),
        'w2p': f32(prop2_W),
    }


_COMPILED = {}


def run_on_cores(inputs, trace=False, **run_kwargs):
    x = np.ascontiguousarray(np.asarray(inputs['x'], dtype=np.float32))
    consts = _host_consts(**{k: v for k, v in inputs.items()
                             if k not in ('x', 'pooling_size')})
    if 'nc' not in _COMPILED:
        _COMPILED['nc'] = build_nc()
    nc = _COMPILED['nc']
    in_maps = []
    for c in range(N_CORES):
        m = {'xs': np.ascontiguousarray(x[c * BPC:(c + 1) * BPC])}
        m.update(consts)
        in_maps.append(m)
    res = run_bass_kernel_spmd(
        nc, in_maps, core_ids=list(range(N_CORES)), trace=trace, **run_kwargs
    )
    out = np.concatenate([res.results[c]['y'] for c in range(N_CORES)], axis=0)
    return out, res


def kernel(**inputs):
    out, _ = run_on_cores(inputs)
    return out


# revision 23
# speedup vs baseline: 1.0851x; 1.0851x over previous
"""Trainium2 Bass kernel for nn_DCGN_78967268704510.

Math: the reference's get_adjacent() builds a diagonal matrix (the faithful
buggy triple loop zeroes every off-diagonal), adds I, then symmetric-
normalizes; for a diagonal matrix D^-1/2 A D^-1/2 == I exactly (to fp32
ulps).  attn_pool feeds only get_adjacent, so the whole network collapses
to two fused stages:

  h   = leaky( (sum_p x[:,4s+p,:] * conv1_w[p,:]) @ prop1_W + prop1_B )
  out = leaky( (sum_p h[:,4t+p,:] * conv2_w[p,:]) @ prop2_W + prop2_B )

Sharding: pure data parallel, batch 64 -> 8 cores x 8 batches each.

v2.1 design (PE-throughput + DMA-overlap oriented):
  - single SP DMA ring with hand-interleaved order: consts/w1rep/b1rep,
    x(0), x(1), w1p[0:8], x(2), w1p[8:16], x(3), stage-2 weights,
    x(4..7) -- every tensor arrives just before its consumer needs it,
    so the PE never starves and the HWDGE FIFO never head-of-line
    blocks.
  - x arrives as one 2MB DMA per (batch, f-half) in a [128, q*4096+f]
    gathered layout: one SBUF tile per half-batch, 4-deep pool.
  - ~28 warmup matmuls keep the PE busy from t~0 so the HAM clock gate
    opens early and never re-throttles (cold PE halves matmul rate).
  - pool selectors stored contiguously per q (strided gpad windows cost
    ~300ns LDWEIGHTS vs ~190ns contiguous).
  - conv1 scale split 5:3 across DVE and GpSimd (measured rates ~1.4 vs
    ~2.8 us per [128,1024] under DMA load).
  - software pipeline: mm1(b-1) / pool(b) / stage2(b-1) / transpose(b)
    interleaved on the PE; scale(b+2) emitted a full iteration ahead.

All replicated weight/bias tiles are host-built exact fp32 (replicating
on-chip through fp32r matmuls costs ~5e-4 relative rounding and blows
the 2e-2 gate: measured rel err 1.93e-2 -> 2.21e-2).
"""
import sys

if '/opt/trn_rl_repo' not in sys.path:
    sys.path.insert(0, '/opt/trn_rl_repo')

import numpy as np

import concourse.bass as bass
import concourse.mybir as mybir
import concourse.tile as tile
from concourse.bass_utils import run_bass_kernel_spmd
from concourse.vector_clock import ScopedClock

N_CORES = 8
B, N, F, HID, NCLASS, P = 64, 512, 2048, 1100, 512, 4
BPC = B // N_CORES          # 8 batches per core
S = N // P                  # 128 stage-1 nodes
T = S // P                  # 32 stage-2 nodes
FT = F // 128               # 16 f-tiles
JT = (HID + 127) // 128     # 9 j-tiles, last has 76 rows
JLAST = HID - 128 * (JT - 1)
MM1_CHUNKS = (384, 384, 332)   # all >=256 so float32r runs at 1 cyc/row

FP32 = mybir.dt.float32
F32R = mybir.dt.float32r


class PatchedTileContext(tile.TileContext):
    """This container's walrus refuses ANY instruction carrying >1 sync
    wait (the TPB EVENTS struct has a single wait slot and the codegen
    won't split).  Split every multi-wait instruction into single-wait
    same-engine nops followed by the instruction with its last wait."""

    def _split_waits(self, inst):
        si = inst.sync_info
        waits = list(si.on_wait) if si and si.on_wait else []
        if len(waits) <= 1:
            return
        for w in waits[:-1]:
            nop = mybir.InstNoOp(
                name=self.nc.get_next_instruction_name(), ins=[], outs=[]
            )
            nop.engine = inst.engine
            nop.sync_info = mybir.SyncInfo(on_wait=[w], on_update=[])
            nop.bass_nofuse = True
            self._add_instruction(nop)
        inst.sync_info = mybir.SyncInfo(
            on_wait=[waits[-1]], on_update=list(si.on_update or [])
        )

    def _commit_instruction(self, inst, lazy_reg_writes=True):
        if inst.engine != mybir.EngineType.Unassigned:
            self._split_waits(inst)
        return super()._commit_instruction(inst, lazy_reg_writes)

    def _drain_and_barrier(self, tick_clock, wait_clock):
        probe = self.nc.sync.nop()
        wait_clock.add_sem_waits(
            probe.ins, ScopedClock({None: tick_clock.global_clock})
        )
        si = probe.ins.sync_info
        waits = list(si.on_wait) if si and si.on_wait else []
        if si and waits:
            probe.ins.sync_info = mybir.SyncInfo(
                on_wait=waits[:1], on_update=list(si.on_update or [])
            )
        for w in waits[1:]:
            n2 = self.nc.sync.nop()
            n2.ins.sync_info = mybir.SyncInfo(on_wait=[w], on_update=[])
        self.nc.sync.drain()
        self.nc.all_engine_barrier()
        assert self.sems is not None
        popped = self.nc._tile_sem_poison_stack.pop()
        assert popped is self._sem_poison
        self.nc.clear_and_free_semaphores(list(self.sems.allocated().values()))
        self.nc.all_engine_barrier()


def build_nc():
    nc = bass.Bass()
    xs_d = nc.dram_tensor('xs', [BPC, N, F], F32R, kind='ExternalInput')
    gq_d = nc.dram_tensor('gq', [128, 512], F32R, kind='ExternalInput')
    g2_d = nc.dram_tensor('g2', [128, 32], F32R, kind='ExternalInput')
    id_d = nc.dram_tensor('ident', [128, 128], F32R, kind='ExternalInput')
    w1rep_d = nc.dram_tensor('w1rep', [128, F], FP32, kind='ExternalInput')
    b1rep_d = nc.dram_tensor('b1rep', [128, HID], FP32, kind='ExternalInput')
    w2rep_d = nc.dram_tensor('w2rep', [128, HID], FP32, kind='ExternalInput')
    b2rep_d = nc.dram_tensor('b2rep', [128, NCLASS], FP32,
                             kind='ExternalInput')
    w1p_d = nc.dram_tensor('w1p', [F, HID], F32R, kind='ExternalInput')
    w2p_d = nc.dram_tensor('w2p', [HID, NCLASS], F32R, kind='ExternalInput')
    y_d = nc.dram_tensor('y', [BPC, T, NCLASS], FP32, kind='ExternalOutput')
    y_flat = y_d.rearrange('b t c -> (b t) c')   # [256, 512]

    with PatchedTileContext(nc) as tc:
        with (
            tc.tile_pool(name='wpool', bufs=1) as wpool,
            tc.tile_pool(name='cpool', bufs=4) as cpool,
            tc.tile_pool(name='xcpool', bufs=2) as xcpool,
            tc.tile_pool(name='xcTpool', bufs=3) as xcTpool,
            tc.tile_pool(name='h2pool', bufs=1) as h2pool,
            tc.tile_pool(name='hcTpool', bufs=1) as hcTpool,
            tc.tile_pool(name='opool', bufs=1) as opool,
            tc.tile_pool(name='pbig', bufs=2, space='PSUM') as pbigpool,
            tc.tile_pool(name='ph', bufs=3, space='PSUM') as phpool,
            tc.tile_pool(name='p2', bufs=1, space='PSUM') as p2pool,
        ):
            # ---- DMA order group 1: tiny consts + stage-1 scale/bias ----
            gq = wpool.tile([128, 512], F32R, tag='gq')
            nc.sync.dma_start(out=gq[:], in_=gq_d[:])
            g2 = wpool.tile([128, 32], F32R, tag='g2')
            nc.sync.dma_start(out=g2[:], in_=g2_d[:])
            ident = wpool.tile([128, 128], F32R, tag='ident')
            nc.sync.dma_start(out=ident[:], in_=id_d[:])
            w1rep = wpool.tile([128, F], FP32, tag='w1rep')
            nc.sync.dma_start(out=w1rep[:], in_=w1rep_d[:])

            # ---- x loads: one 2MB DMA per (batch, f-half) ----
            cs = {}

            def load_ch(b, h):
                src = xs_d[b].rearrange('(q p) (h f) -> p h q f', p=128, h=2)
                c = cpool.tile([128, 4096], F32R, tag='c',
                               name=f'c_{b}_{h}')
                nc.sync.dma_start(
                    out=c.rearrange('p (q f) -> p q f', q=4),
                    in_=src[:, h],
                )
                cs[(b, h)] = c

            def load_c(b):
                load_ch(b, 0)
                load_ch(b, 1)

            b1rep = wpool.tile([128, HID], FP32, tag='b1rep')
            nc.sync.dma_start(out=b1rep[:], in_=b1rep_d[:])

            load_c(0)
            load_c(1)

            # ---- DMA order group 2: first chunk of w1p ----
            w1ps = []
            for k in range(FT):
                w1ps.append(wpool.tile([128, HID], F32R, tag=f'w1p{k}',
                                       name=f'w1p{k}'))

            def load_w1p(k0, k1):
                for k in range(k0, k1):
                    nc.sync.dma_start(
                        out=w1ps[k][:], in_=w1p_d[k * 128:(k + 1) * 128, :]
                    )

            load_w1p(0, 6)

            # stage-2 weights (loaded at iteration 1)
            w2rep = wpool.tile([128, HID], FP32, tag='w2rep')
            b2rep = wpool.tile([128, NCLASS], FP32, tag='b2rep')
            w2p = wpool.tile([128, JT * NCLASS], F32R, tag='w2p')

            def load_w2():
                nc.sync.dma_start(out=w2rep[:], in_=w2rep_d[:])
                for m in range(JT):
                    rows = 128 if m < JT - 1 else JLAST
                    nc.sync.dma_start(
                        out=w2p[0:rows, m * NCLASS:(m + 1) * NCLASS],
                        in_=w2p_d[m * 128:m * 128 + rows, :],
                    )
                nc.sync.dma_start(out=b2rep[:], in_=b2rep_d[:])

            # ---- PE warmup: dummy matmuls during the initial DMA wait so
            #      the HAM clock gate reaches K=8/8 before batch 0 ----
            for w in range(4):
                warm_ps = p2pool.tile([128, 512], FP32, tag='p2',
                                      name=f'warm{w}')
                for i in range(7):
                    nc.tensor.matmul(warm_ps[:], ident[:], gq[:],
                                     start=(i == 0), stop=(i == 6))

            # ---- per-batch stages ----
            # ALL elementwise work runs on DVE: any concurrent GpSimd
            # tensor op degrades DVE from ~1.2us to ~3.4us per [128,1024]
            # (measured), so GpSimd's 3.3us/op "help" is net-negative.

            def scale_c(b):
                for h in range(2):
                    cv = cs[(b, h)].rearrange('p (q f) -> p q f', q=4)
                    wsl = w1rep[:, h * 1024:(h + 1) * 1024]
                    for q in range(4):
                        nc.vector.tensor_mul(cv[:, q], cv[:, q], wsl)

            xcs = {}
            xcTs = {}
            h2s = {}
            hcT = [None, None]

            def pool_half(b, h):
                """pool f-half h of batch b -> xc[b][h] (SBUF [128,1024])."""
                cv = cs[(b, h)].rearrange('p (q f) -> p q f', q=4)
                pp = pbigpool.tile([128, 1024], FP32, tag='pbig',
                                   name=f'pp_{b}_{h}')
                for c2 in range(2):
                    for q in range(4):
                        nc.tensor.matmul(
                            pp[:, 512 * c2:512 * (c2 + 1)],
                            gq[:, 128 * q:128 * (q + 1)],
                            cv[:, q, 512 * c2:512 * (c2 + 1)],
                            start=(q == 0), stop=(q == 3),
                        )
                xc = xcpool.tile([128, 1024], F32R, tag='xc',
                                 name=f'xc_{b}_{h}')
                nc.scalar.copy(out=xc[:], in_=pp[:])
                xcs[(b, h)] = xc

            def transpose_half(b, h):
                xc = xcs.pop((b, h))
                pt = pbigpool.tile([128, 1024], F32R, tag='pbig',
                                   name=f'pt_{b}_{h}')
                for kk in range(8):
                    nc.tensor.transpose(
                        pt[:, 128 * kk:128 * (kk + 1)],
                        xc[:, kk * 128:(kk + 1) * 128],
                        ident[:],
                    )
                xcT = xcTpool.tile([128, 1024], F32R, tag='xcT',
                                   name=f'xcT_{b}_{h}')
                nc.scalar.copy(out=xcT[:], in_=pt[:])
                xcTs[(b, h)] = xcT

            def mm1(b):
                """mm1 + bias + leaky + conv2-scale for batch b."""
                xcT0 = xcTs.pop((b, 0))
                xcT1 = xcTs.pop((b, 1))
                xcT = (xcT0, xcT1)
                h2 = h2pool.tile([128, HID], F32R, tag='h2', name=f'h2_{b}')
                c0 = 0
                for cn in MM1_CHUNKS:
                    pht = phpool.tile([128, cn], FP32, tag='ph',
                                      name=f'ph_{b}_{c0}')
                    for k in range(FT):
                        nc.tensor.matmul(
                            pht[:],
                            xcT[k // 8][:, (k % 8) * 128:(k % 8 + 1) * 128],
                            w1ps[k][:, c0:c0 + cn],
                            start=(k == 0), stop=(k == FT - 1),
                        )
                    nc.vector.tensor_add(
                        h2[:, c0:c0 + cn], pht[:], b1rep[:, c0:c0 + cn]
                    )
                    nc.scalar.activation(
                        h2[:, c0:c0 + cn], h2[:, c0:c0 + cn],
                        mybir.ActivationFunctionType.Lrelu, alpha=0.01,
                    )
                    c0 += cn
                nc.vector.tensor_mul(h2[:], h2[:], w2rep[:])
                h2s[b] = h2

            def stage2(b):
                """stage-2 pool-transpose + (every 4th) mm2 + store."""
                h2 = h2s.pop(b)
                pt2 = p2pool.tile([128, JT * T], FP32, tag='p2',
                                  name=f'pt2_{b}')
                for m in range(JT):
                    rows = 128 if m < JT - 1 else JLAST
                    nc.tensor.matmul(
                        pt2[0:rows, m * T:(m + 1) * T],
                        h2[:, m * 128:m * 128 + rows],
                        g2[:],
                        start=True, stop=True,
                    )
                g, bg = divmod(b, 4)
                if bg == 0:
                    hcT[g] = hcTpool.tile(
                        [128, JT * 128], F32R, tag='hcT', name=f'hcT{g}'
                    )
                dst = hcT[g].rearrange('p (m c) -> p m c', m=JT)[
                    :, :, 32 * bg:32 * (bg + 1)
                ]
                src = pt2[:].rearrange('p (m c) -> p m c', m=JT)
                # region-exact: rows [JLAST:128] of the last j-block are
                # never written by the pt2 matmuls
                nc.scalar.copy(out=dst[:, 0:JT - 1], in_=src[:, 0:JT - 1])
                nc.scalar.copy(out=dst[0:JLAST, JT - 1:JT],
                               in_=src[0:JLAST, JT - 1:JT])

                if bg == 3:
                    po = p2pool.tile([128, NCLASS], FP32, tag='p2',
                                     name=f'po_{g}')
                    for m in range(JT):
                        rows = 128 if m < JT - 1 else JLAST
                        nc.tensor.matmul(
                            po[:],
                            hcT[g][0:rows, m * 128:(m + 1) * 128],
                            w2p[0:rows, m * NCLASS:(m + 1) * NCLASS],
                            start=(m == 0), stop=(m == JT - 1),
                        )
                    ob = opool.tile([128, NCLASS], FP32, tag='ob',
                                    name=f'ob_{g}')
                    nc.vector.tensor_add(ob[:], po[:], b2rep[:])
                    nc.scalar.activation(
                        ob[:], ob[:],
                        mybir.ActivationFunctionType.Lrelu, alpha=0.01,
                    )
                    nc.sync.dma_start(
                        out=y_flat[128 * g:128 * (g + 1), :], in_=ob[:]
                    )

            # ---- software pipeline ----
            scale_c(0)
            scale_c(1)
            for b in range(BPC):
                if b == 0:
                    # interleave the rest of w1p with x(2) at half-batch
                    # granularity so neither stream starves its consumer
                    load_ch(2, 0)
                    load_w1p(6, 11)
                    load_ch(2, 1)
                    load_w1p(11, FT)
                elif b + 2 < BPC:
                    load_c(b + 2)
                if b == 1:
                    load_w2()
                if b >= 1:
                    mm1(b - 1)
                pool_half(b, 0)
                pool_half(b, 1)
                transpose_half(b, 0)
                transpose_half(b, 1)
                if b >= 1:
                    stage2(b - 1)
                if b + 2 < BPC:
                    scale_c(b + 2)
                cs.pop((b, 0))
                cs.pop((b, 1))
            mm1(BPC - 1)
            stage2(BPC - 1)
    return nc


def _host_consts(conv1_w, pool1_w, pool1_b, prop1_W, prop1_B,
                 conv2_w, pool2_w, pool2_b, prop2_W, prop2_B):
    f32 = lambda a: np.ascontiguousarray(np.asarray(a, dtype=np.float32))
    gq = np.zeros((128, 512), dtype=np.float32)
    n = np.arange(128)
    for q in range(4):
        gq[n, 128 * q + 32 * q + n // 4] = 1.0
    g2 = np.zeros((128, 32), dtype=np.float32)
    g2[n, n // 4] = 1.0
    return {
        'gq': gq,
        'g2': g2,
        'ident': np.eye(128, dtype=np.float32),
        'w1rep': f32(np.tile(np.asarray(conv1_w), (32, 1))),
        'b1rep': f32(np.broadcast_to(np.asarray(prop1_B), (128, HID))),
        'w2rep': f32(np.tile(np.asarray(conv2_w), (32, 1))),
        'b2rep': f32(np.broadcast_to(np.asarray(prop2_B), (128, NCLASS))),
        'w1p': f32(prop1_W),
        'w2p': f32(prop2_W),
    }


_COMPILED = {}


def run_on_cores(inputs, trace=False, **run_kwargs):
    x = np.ascontiguousarray(np.asarray(inputs['x'], dtype=np.float32))
    consts = _host_consts(**{k: v for k, v in inputs.items()
                             if k not in ('x', 'pooling_size')})
    if 'nc' not in _COMPILED:
        _COMPILED['nc'] = build_nc()
    nc = _COMPILED['nc']
    in_maps = []
    for c in range(N_CORES):
        m = {'xs': np.ascontiguousarray(x[c * BPC:(c + 1) * BPC])}
        m.update(consts)
        in_maps.append(m)
    res = run_bass_kernel_spmd(
        nc, in_maps, core_ids=list(range(N_CORES)), trace=trace, **run_kwargs
    )
    out = np.concatenate([res.results[c]['y'] for c in range(N_CORES)], axis=0)
    return out, res


def kernel(**inputs):
    out, _ = run_on_cores(inputs)
    return out
